# revision 62
# baseline (speedup 1.0000x reference)
"""Bass/Trainium2 kernel for BiasedAttention (B=8, N=2048, H=256), SPMD over 8 cores.

Per-core work (data-parallel over batch): one batch element.
  Q = x@Wq*s + bq*s ; K = x@Wk + bk ; V = x@(Wv Wo)   (b_V Wo + b_O folded into bo)
  S = Q K^T + attn_bias ; P = exp(S) (unnormalized)
  O = (P @ [V|1]) -> numerator and denominator in one matmul ; out = O/den + bo

Q^T/K^T are kept in fp8e4m3 so the score matmuls run in DoubleRow perf mode
(contracts all 256 h in one matmul at 0.5 cycles/row). P/V stay bf16 (fp8
there fails the 2e-2 tolerance). PSUM evacuations are split across ACT/DVE;
SBUF-only elementwise work goes to Pool (gpsimd), which has no PSUM port.
"""

import contextlib
import sys

for _p in ("/opt/trn_rl_repo", "/root/.axon_site/_ro/trn_rl_repo"):
    if _p not in sys.path:
        sys.path.append(_p)

import numpy as np

import concourse.bass as bass
import concourse.tile as tile
from concourse import mybir
from concourse.bass_utils import run_bass_kernel_spmd
from concourse.vector_clock import ScopedClock

B, N, H = 8, 2048, 256
SCALE = H ** -0.5
P = 128
NT = N // P          # 16 row tiles per core
HC = H // P          # 2 h chunks
KC = N // 512        # 4 k chunks of 512
F32 = mybir.dt.float32
FP16 = mybir.dt.float16
BF16 = mybir.dt.bfloat16
FP8 = mybir.dt.float8e4
DR = mybir.MatmulPerfMode.DoubleRow
USE_DR = True  # fp8 DoubleRow for the score matmuls (else bf16)


def _patch_tile_drain():
    """walrus here rejects >1 sync-wait on a CTRL/Drain instruction; split the
    TileContext exit-drain's waits across a chain of drains."""
    if getattr(tile.TileContext, "_drain_patched", False):
        return

    def _drain_and_barrier(self, tick_clock, wait_clock):
        drain_inst = self.nc.sync.drain()
        wait_clock.add_sem_waits(
            drain_inst.ins, ScopedClock({None: tick_clock.global_clock})
        )
        si = drain_inst.ins.sync_info
        waits = list(si.on_wait) if si is not None and si.on_wait else []
        if len(waits) > 1:
            drain_inst.ins.sync_info = mybir.SyncInfo(on_wait=waits[:1], on_update=[])
            engs = [self.nc.sync, self.nc.vector, self.nc.scalar,
                    self.nc.tensor, self.nc.gpsimd]
            for i, w in enumerate(waits[1:]):
                d2 = engs[i % len(engs)].drain()
                d2.ins.sync_info = mybir.SyncInfo(on_wait=[w], on_update=[])
        self.nc.all_engine_barrier()
        assert self.sems is not None
        popped = self.nc._tile_sem_poison_stack.pop()
        assert popped is self._sem_poison
        self.nc.clear_and_free_semaphores(list(self.sems.allocated().values()))
        self.nc.all_engine_barrier()

    tile.TileContext._drain_and_barrier = _drain_and_barrier
    tile.TileContext._drain_patched = True


MAX_SYNC_WAITS = 1


def _split_sync_waits(nc: bass.Bass, max_waits: int = MAX_SYNC_WAITS):
    """walrus rejects instructions with too many sync waits; hoist the excess
    onto same-engine NOPs inserted just before the instruction."""
    for fn in nc.m.functions:
        for bb in fn.blocks:
            new = []
            for inst in bb.instructions:
                si = inst.sync_info
                waits = list(si.on_wait) if si is not None and si.on_wait else []
                if len(waits) > max_waits:
                    inst.sync_info = mybir.SyncInfo(
                        on_wait=waits[-max_waits:],
                        on_update=list(si.on_update) if si.on_update else [],
                    )
                    excess = waits[:-max_waits]
                    for i in range(0, len(excess), max_waits):
                        nop = mybir.InstNoOp(
                            name=nc.get_next_instruction_name(),
                            sync_info=mybir.SyncInfo(
                                on_wait=excess[i:i + max_waits], on_update=[]
                            ),
                            engine=inst.engine,
                            bass_nofuse=True,
                        )
                        new.append(nop)
                new.append(inst)
            bb.instructions[:] = new


def build_program(repeat: int = 1) -> bass.Bass:
    _patch_tile_drain()
    nc = bass.Bass()
    Exp = mybir.ActivationFunctionType.Exp

    x_d = nc.declare_dram_parameter("x", [N, H], F32, isOutput=False)
    ab_d = nc.declare_dram_parameter("ab", [N, N], F32, isOutput=False)
    w_d = {
        k: nc.declare_dram_parameter(k, [H, H], F32, isOutput=False)
        for k in ("wq", "wk", "wv")
    }
    b_d = {
        k: nc.declare_dram_parameter(k, [1, H], F32, isOutput=False)
        for k in ("bq", "bk", "bo")
    }
    id_d = nc.declare_dram_parameter("ident", [P, P], F32, isOutput=False)
    y_d = nc.declare_dram_parameter("y", [N, H], F32, isOutput=True)

    with tile.TileContext(nc) as tc:
        with (
            tc.tile_pool(name="const", bufs=1) as const,
            tc.tile_pool(name="wstage", bufs=3) as wstage,
            tc.tile_pool(name="rowstage", bufs=2) as rowstage,
            tc.tile_pool(name="acts", bufs=1) as acts,
            tc.tile_pool(name="bias", bufs=5) as biasp,
            tc.tile_pool(name="s", bufs=3) as sp,
            tc.tile_pool(name="ptsb", bufs=3) as ptsb,
            tc.tile_pool(name="small", bufs=4) as small,
            tc.tile_pool(name="ysb", bufs=3) as ysb,
            tc.tile_pool(name="qk", bufs=4, space="PSUM") as qkp,
            tc.tile_pool(name="pt", bufs=2, space="PSUM") as ptp,
            tc.tile_pool(name="o", bufs=2, space="PSUM") as op_,
        ):
            loop_cm = (
                tc.For_i(0, repeat, 1) if repeat > 1 else contextlib.nullcontext()
            )
            with loop_cm:
                # ---- SP queue: identity (tiny, gates first transposes) then
                # x; first x tile alone so the transpose chain starts ASAP ----
                id_f32 = const.tile([P, P], F32)
                nc.sync.dma_start(out=id_f32[:], in_=id_d[:])
                x_sb = acts.tile([P, NT, H], F32, name="x_sb")
                x_re = x_d.rearrange("(t p) h -> p t h", p=P)
                for sl in (slice(0, 1), slice(1, 4), slice(4, 8),
                           slice(8, 12), slice(12, 16)):
                    nc.sync.dma_start(out=x_sb[:, sl, :], in_=x_re[:, sl, :])
                wf_st = {}
                for k in ("wq", "wk", "wv"):
                    wf = wstage.tile([P, HC, H], F32, name=f"wf_{k}")
                    nc.scalar.dma_start(
                        out=wf[:], in_=w_d[k].rearrange("(c p) o -> p c o", p=P)
                    )
                    wf_st[k] = wf
                bcol = {}
                for k in ("bq", "bk"):
                    bc_ = const.tile([P, HC], F32, name=f"{k}_col")
                    nc.scalar.dma_start(
                        out=bc_[:], in_=b_d[k].rearrange("a (c p) -> p (a c)", p=P)
                    )
                    bcol[k] = bc_
                bo_row = rowstage.tile([1, H], F32, name="bo_row")
                nc.scalar.dma_start(out=bo_row[:], in_=b_d["bo"][:])

                # ---- bias pairs: eager issue in CONSUMPTION order (so the
                # bufs=5 reuse dependency is pair pr -> pair pr-5), spread
                # over the SP and ACT HWDGE queues (ready DMAs pass waiters)
                ab_re = ab_d.rearrange("(pr t p) k -> pr p t k", pr=NT // 2, p=P)
                bias_pairs = [None] * (NT // 2)
                for pr in range(NT // 2):
                    bias_pairs[pr] = biasp.tile([P, 2, N], F32, name="bias")
                    eng = nc.sync if pr % 2 == 0 else nc.scalar
                    eng.dma_start(out=bias_pairs[pr][:], in_=ab_re[pr])

                # ---- converts: id on DVE (gates first transposes); x_bf g0
                # on DVE (critical), rest on Pool; weight converts on ACT ----
                id_bf = const.tile([P, P], BF16)
                nc.vector.tensor_copy(id_bf[:], id_f32[:])
                id_fp = const.tile([P, P], FP16)
                nc.gpsimd.tensor_copy(id_fp[:], id_f32[:])
                x_bf = acts.tile([P, NT, H], BF16, name="x_bf")
                nc.vector.tensor_copy(x_bf[:, 0:1, :], x_sb[:, 0:1, :])
                nc.vector.tensor_copy(x_bf[:, 1:4, :], x_sb[:, 1:4, :])
                for tg in range(1, 4):
                    sl = slice(tg * 4, (tg + 1) * 4)
                    nc.gpsimd.tensor_copy(x_bf[:, sl, :], x_sb[:, sl, :])
                wsb = {}
                for k in ("wq", "wk", "wv"):
                    wb = const.tile([P, HC, H], BF16, name=f"{k}_bf")
                    nc.scalar.copy(wb[:], wf_st[k][:])
                    wsb[k] = wb

                # ---- x^T (bf16, [h part, hc, n]) via PE identity matmuls ----
                xt = acts.tile([P, HC, N], BF16, name="xt")

                def emit_xt_group(tg):
                    for hc in range(HC):
                        ps = qkp.tile([P, 512], F32, name="qk")
                        for j in range(4):
                            t = tg * 4 + j
                            nc.tensor.matmul(
                                ps[:, j * P:(j + 1) * P],
                                lhsT=x_bf[:, t, hc * P:(hc + 1) * P],
                                rhs=id_bf[:],
                                start=True, stop=True,
                            )
                        nc.vector.tensor_copy(xt[:, hc, tg * 512:(tg + 1) * 512], ps[:])

                # ---- Q^T, K^T (fp8e4m3, [h_out part, hc, n]) for DoubleRow ----
                qk_dt = FP8 if USE_DR else BF16
                qt = acts.tile([P, HC, N], qk_dt, name="qt")
                kt = acts.tile([P, HC, N], qk_dt, name="kt")

                def emit_qtkt_ng(ng):
                    for name_, dst, wkey, bkey in (
                        ("qt", qt, "wq", "bq"), ("kt", kt, "wk", "bk"),
                    ):
                        for ho in range(HC):
                            ps = qkp.tile([P, 512], F32, name="qk")
                            for hi in range(HC):
                                nc.tensor.matmul(
                                    ps[:],
                                    lhsT=wsb[wkey][:, hi, ho * P:(ho + 1) * P],
                                    rhs=xt[:, hi, ng * 512:(ng + 1) * 512],
                                    start=(hi == 0), stop=(hi == HC - 1),
                                )
                            if name_ == "qt":
                                nc.scalar.activation(
                                    dst[:, ho, ng * 512:(ng + 1) * 512], ps[:],
                                    mybir.ActivationFunctionType.Identity,
                                    bias=bcol[bkey][:, ho:ho + 1],
                                )
                            else:
                                nc.vector.tensor_scalar_add(
                                    dst[:, ho, ng * 512:(ng + 1) * 512], ps[:],
                                    bcol[bkey][:, ho:ho + 1],
                                )

                # ---- V_ext (bf16, [n part, t, h | ones]); b_V folded out ----
                v_sb = acts.tile([P, NT, H + 1], BF16, name="v")
                nc.vector.memset(v_sb[:, :, H:H + 1], 1.0)

                def emit_v_chunk(t):
                    ps = qkp.tile([P, 512], F32, name="qk")
                    for hi in range(HC):
                        nc.tensor.matmul(
                            ps[:, :H],
                            lhsT=xt[:, hi, t * P:(t + 1) * P],
                            rhs=wsb["wv"][:, hi, :],
                            start=(hi == 0), stop=(hi == HC - 1),
                        )
                    if t % 2 == 0:
                        nc.scalar.copy(v_sb[:, t, :H], ps[:, :H])
                    else:
                        nc.vector.tensor_copy(v_sb[:, t, :H], ps[:, :H])

                # b_O broadcast across partitions via K=1 ones-matmul
                ones_f = const.tile([1, P], F32, name="ones_f")
                nc.vector.memset(ones_f[:], 1.0)
                bo_bc = const.tile([P, H], F32, name="bo_bc")

                # ---- main loop over q tiles, software-pipelined 3 deep ----
                # iteration i issues: scores+bias-add for tile i, transpose+
                # exp for tile i-1, PV+normalize for tile i-2. Cross-engine
                # consumers thus trail their producers by a full iteration.
                y_re = y_d.rearrange("(gr t p) h -> gr p t h", gr=NT // 4, p=P)
                s_tiles = [None] * NT
                pt_tiles = [None] * NT
                y_group = [None]

                def emit_score_chunk(qt_i, kc):
                    pr, sub = divmod(qt_i, 2)
                    b_t = bias_pairs[pr][:, sub, :]
                    if kc == 0:
                        s_tiles[qt_i] = sp.tile([P, N], FP16, name="s")
                    s_t = s_tiles[qt_i]
                    ps_s = qkp.tile([P, 512], F32, name="qk")
                    if USE_DR:
                        nc.tensor.matmul(
                            ps_s[:],
                            lhsT=qt[:, :, qt_i * P:(qt_i + 1) * P],
                            rhs=kt[:, :, kc * 512:(kc + 1) * 512],
                            start=True, stop=True,
                            perf_mode=DR,
                        )
                    else:
                        for hi in range(HC):
                            nc.tensor.matmul(
                                ps_s[:],
                                lhsT=qt[:, hi, qt_i * P:(qt_i + 1) * P],
                                rhs=kt[:, hi, kc * 512:(kc + 1) * 512],
                                start=(hi == 0), stop=(hi == HC - 1),
                            )
                    nc.vector.tensor_add(
                        s_t[:, kc * 512:(kc + 1) * 512],
                        ps_s[:],
                        b_t[:, kc * 512:(kc + 1) * 512],
                    )

                def stage_scores(qt_i):
                    for kc in range(KC):
                        emit_score_chunk(qt_i, kc)

                def emit_transpose_group(qt_i, g):
                    # transpose scores (fp16); exp during the PSUM->SBUF
                    # evacuation on ACT: pt = exp(s^T)
                    s_t = s_tiles[qt_i]
                    if g == 0:
                        pt_tiles[qt_i] = ptsb.tile([P, NT, P], BF16, name="pt")
                    pt_t = pt_tiles[qt_i]
                    ps_pt = ptp.tile([P, 512], F32, name="pt_ps")
                    for j in range(4):
                        kc4 = g * 4 + j
                        nc.tensor.matmul(
                            ps_pt[:, j * P:(j + 1) * P],
                            lhsT=s_t[:, kc4 * P:(kc4 + 1) * P],
                            rhs=id_fp[:],
                            start=True, stop=True,
                        )
                    nc.scalar.activation(
                        pt_t[:, g * 4:(g + 1) * 4, :], ps_pt[:], Exp,
                    )

                def stage_transpose(qt_i):
                    for g in range(4):
                        emit_transpose_group(qt_i, g)

                def emit_pv_quartet(qt_i, q4, ps_o):
                    pt_t = pt_tiles[qt_i]
                    for kc16 in range(q4 * 4, q4 * 4 + 4):
                        nc.tensor.matmul(
                            ps_o[:, :H + 1],
                            lhsT=pt_t[:, kc16, :],
                            rhs=v_sb[:, kc16, :],
                            start=(kc16 == 0), stop=(kc16 == NT - 1),
                        )

                def finish_pv(qt_i, ps_o):
                    rden = small.tile([P, 1], F32, name="rden")
                    nc.vector.reciprocal(rden[:], ps_o[:, H:H + 1])
                    y1 = small.tile([P, H], F32, name="y1")
                    nc.scalar.activation(
                        y1[:], ps_o[:, :H], mybir.ActivationFunctionType.Copy,
                        scale=rden[:],
                    )
                    if qt_i % 4 == 0:
                        y_group[0] = ysb.tile([P, 4, H], F32, name="y")
                    badd = nc.vector if qt_i >= NT - 4 else nc.gpsimd
                    badd.tensor_add(y_group[0][:, qt_i % 4, :], y1[:], bo_bc[:])
                    if qt_i == NT - 3:
                        # split the last output group so its DMA starts early
                        nc.sync.dma_start(
                            out=y_re[qt_i // 4][:, 0:2, :],
                            in_=y_group[0][:, 0:2, :],
                        )
                    elif qt_i == NT - 1:
                        nc.sync.dma_start(
                            out=y_re[qt_i // 4][:, 2:4, :],
                            in_=y_group[0][:, 2:4, :],
                        )
                    elif qt_i % 4 == 3:
                        nc.sync.dma_start(
                            out=y_re[qt_i // 4], in_=y_group[0][:],
                        )

                # prologue
                for ng in range(KC):
                    emit_xt_group(ng)
                    emit_qtkt_ng(ng)

                pv_tiles = {}

                def stage_pv(qt_i):
                    ps_o = op_.tile([P, 512], F32, name="o")
                    for q4 in range(4):
                        emit_pv_quartet(qt_i, q4, ps_o)
                    finish_pv(qt_i, ps_o)

                for i in range(NT + 2):
                    if i < NT and i >= 1:
                        # interleave score chunks (tile i) with transpose
                        # groups (tile i-1): spaces out PSUM buffer demand
                        for kc in range(KC):
                            emit_score_chunk(i, kc)
                            emit_transpose_group(i - 1, kc)
                    elif i < NT:
                        stage_scores(i)
                    elif i == NT:
                        stage_transpose(i - 1)
                    if i == 0:
                        # V projections ride in the pipeline prologue: PE does
                        # them while DVE runs tile-0 bias adds
                        for t in range(10):
                            emit_v_chunk(t)
                    if i == 1:
                        for t in range(10, NT):
                            emit_v_chunk(t)
                        ps_bo = qkp.tile([P, 512], F32, name="qk")
                        nc.tensor.matmul(ps_bo[:, :H], lhsT=ones_f[:],
                                         rhs=bo_row[:], start=True, stop=True)
                        nc.vector.tensor_copy(bo_bc[:], ps_bo[:, :H])
                    if i >= 2:
                        stage_pv(i - 2)

    _split_sync_waits(nc)
    return nc


_NC = None


def _get_program():
    global _NC
    if _NC is None:
        _NC = build_program()
    return _NC


def make_in_maps(x, attn_bias, W_Q, b_Q, W_K, b_K, W_V, b_V, W_O, b_O):
    f = np.float32
    W_V, W_O = np.asarray(W_V, np.float64), np.asarray(W_O, np.float64)
    b_V = np.asarray(b_V, np.float64)
    shared = {
        "wq": np.ascontiguousarray(np.asarray(W_Q, f) * SCALE),
        "wk": np.ascontiguousarray(np.asarray(W_K, f)),
        # W_O folded into the V projection: y = (P @ [V Wo | 1])/den + bo
        "wv": np.ascontiguousarray((W_V @ W_O).astype(f)),
        "bq": np.asarray(b_Q, f).reshape(1, H) * SCALE,
        "bk": np.asarray(b_K, f).reshape(1, H),
        # b_V folded through W_O into the output bias
        "bo": ((b_V @ W_O).astype(f) + np.asarray(b_O, f)).reshape(1, H),
        "ident": np.eye(P, dtype=f),
    }
    x = np.asarray(x, f)
    ab = np.asarray(attn_bias, f)
    return [
        {"x": np.ascontiguousarray(x[b]), "ab": np.ascontiguousarray(ab[b]), **shared}
        for b in range(B)
    ]


def kernel(x, attn_bias, W_Q, b_Q, W_K, b_K, W_V, b_V, W_O, b_O, _trace=False):
    nc = _get_program()
    in_maps = make_in_maps(x, attn_bias, W_Q, b_Q, W_K, b_K, W_V, b_V, W_O, b_O)
    res = run_bass_kernel_spmd(nc, in_maps, core_ids=list(range(B)), trace=_trace)
    out = np.stack([res.results[b]["y"] for b in range(B)], axis=0)
    if _trace:
        kernel.last_results = res
    return out
